# revision 9
# baseline (speedup 1.0000x reference)
"""Trainium2 Bass kernel for nn_Attention (dense transformer attention block).

Reference computation (shapes fixed):
  x [2, 256, 48, 48] -> RMSNorm over channels -> 1x1 conv to qkv (8 heads, 64 dhead)
  -> prepend 4 learnable mem kv tokens -> softmax attention -> 1x1 conv out [2, 256, 48, 48]

Sharding: 8 cores = 2 batches x 4 head-pairs. Core c handles batch c//4 and
heads (2g, 2g+1), g = c%4. Each core computes its heads' normalized attention
output oT [128 = 2x64 dhead, q]; per query-chunk the 4 cores of a batch
AllGather their oT (bf16) and every core applies the out-projection locally
for its own 64 output channels -> no reduce collective, tiny tail.

Numerics: qkv projection in float32r; attention matmuls bf16 with fp32 psum.
Layout highlights:
  - x, xn in [channel, pos]; RMSNorm scale sinv broadcast to all partitions
    via all-ones-lhsT matmul of x^2; sinv folded into xn before projections.
  - q/k in [dhead(2 heads packed), pos]; sim row-packed (head A on PE rows
    0-63, head B on 64-127, concurrent via row groups), scores S^T [key, q].
  - v^T computed directly by matmul (lhsT = xn pos-tile, rhs = w_v cols),
    no PE transposes.
  - attn@v per head: lhsT [ones | zeros*63 | v] -> denom on psum row 0,
    out^T on rows 64:128 (both heads identical; no partition-crossing ops).
    Normalized outputs go out as oT2 [rows 64:128, head, q]; the AllGather
    payload is [64, 2, q] per rank and the gather-back DMAs place each
    rank's two heads onto partitions 0:64 / 64:128 of the rhs tiles.
"""
import numpy as np

import concourse.mybir as mybir
import concourse.tile as tile
from concourse import bacc
from concourse.bass_utils import run_bass_kernel_spmd

F32 = mybir.dt.float32
F32R = mybir.dt.float32r
BF16 = mybir.dt.bfloat16
EXP = mybir.ActivationFunctionType.Exp
SQRT = mybir.ActivationFunctionType.Sqrt

DIM = 256
HEADS = 8
DHEAD = 64
MEM = 4
HID = 512
N = 48 * 48          # 2304 image positions
NJT = N // 128       # 18 image j-tiles; j-tile NJT holds the 4 mem tokens
GROUPS = [[0, 1, 2, 3], [4, 5, 6, 7]]

# i-chunks of the query axis (last chunk smallest -> smallest tail)
CHUNKS = [(0, 512), (512, 512), (1024, 512), (1536, 512), (2048, 256)]


def build():
    nc = bacc.Bacc("TRN2", target_bir_lowering=False, debug=False,
                   enable_asserts=True, num_devices=8)
    x_d = nc.dram_tensor("x", [DIM, N], F32, kind="ExternalInput").ap()
    wqkv_d = nc.dram_tensor("wqkv", [DIM, 384], F32, kind="ExternalInput").ap()
    memk_d = nc.dram_tensor("memk", [128, MEM], F32, kind="ExternalInput").ap()
    memv_d = nc.dram_tensor("memv", [MEM, 2, DHEAD], F32, kind="ExternalInput").ap()
    wout_d = nc.dram_tensor("wout", [4, 128, DHEAD], F32, kind="ExternalInput").ap()
    out_d = nc.dram_tensor("out", [DHEAD, N], F32, kind="ExternalOutput").ap()

    NCH = len(CHUNKS)
    with tile.TileContext(nc) as tc:
        with (
            tc.tile_pool(name="consts", bufs=1) as consts,
            tc.tile_pool(name="big", bufs=1) as big,
            tc.tile_pool(name="io", bufs=2) as io,
            tc.tile_pool(name="pP", bufs=4) as pP,
            tc.tile_pool(name="gth", bufs=2) as gth,
            tc.tile_pool(name="ps_s", bufs=2, space="PSUM") as ps_s,
            tc.tile_pool(name="ps_a", bufs=2, space="PSUM") as ps_a,
            tc.tile_pool(name="dram", bufs=1, space="DRAM") as dram,
        ):
            # ---------------- constants ----------------
            ones_f = consts.tile([128, 1], F32)
            nc.vector.memset(ones_f[:, :], 1.0)
            zeros_f = consts.tile([128, 1], F32)
            nc.vector.memset(zeros_f[:, :], 0.0)
            ones_r = consts.tile([128, 128], F32R)
            nc.vector.tensor_copy(ones_r[:, :], ones_f[:, :].to_broadcast((128, 128)))

            # ---------------- collective warmup ----------------
            # first collective on a NEFF pays ~30-60us firmware cold start;
            # absorb it behind the compute phase with a tiny dummy.
            warm_sb = consts.tile([1, 32], F32)
            nc.vector.memset(warm_sb[:, :], 0.0)
            wi = dram.tile([1, 32], F32, tag="wi")
            wo = dram.tile([1, 32], F32, tag="wo")
            nc.sync.dma_start(out=wi[:, :], in_=warm_sb[:, :])
            nc.gpsimd.collective_compute(
                "AllReduce", mybir.AluOpType.add,
                replica_groups=GROUPS,
                ins=[wi[:, :].opt()],
                outs=[wo[:, :].opt()],
            )

            # ---------------- vT tile skeletons ----------------
            # per (head, jt): [key, 128]: col 0 = ones, 1:64 zeros,
            # 64:128 = v^T.  jt == NJT holds the 4 mem tokens on rows 0:4.
            vT = [[None, None] for _ in range(NJT + 1)]
            for jt in range(NJT + 1):
                for h in range(2):
                    t = big.tile([128, 128], BF16, tag=f"vT{h}_{jt}")
                    vT[jt][h] = t
                    nc.vector.tensor_copy(
                        t[:, 0:1], ones_f[:, :].to_broadcast((128, 1)))
                    nc.vector.tensor_copy(
                        t[:, 1:64], zeros_f[:, :].to_broadcast((128, 63)))

            # ---------------- load inputs ----------------
            xs = [[None] * NCH, [None] * NCH]
            dma_engines = [nc.sync, nc.scalar, nc.gpsimd]
            for ci, (c0, cw) in enumerate(CHUNKS):
                for kt in range(2):
                    t = big.tile([128, cw], F32, tag=f"x{kt}_{ci}")
                    xs[kt][ci] = t
                    eng = dma_engines[(2 * ci + kt) % 3]
                    eng.dma_start(
                        out=t[:, :], in_=x_d[128 * kt:128 * kt + 128, c0:c0 + cw])

            wq_f = io.tile([128, 2, 384], F32)
            nc.sync.dma_start(out=wq_f[:, 0, :], in_=wqkv_d[0:128, :])
            nc.sync.dma_start(out=wq_f[:, 1, :], in_=wqkv_d[128:256, :])
            wq = consts.tile([128, 2, 384], F32R)
            nc.vector.tensor_copy(wq[:, :, :], wq_f[:, :, :])

            memk_f = io.tile([128, MEM], F32)
            nc.sync.dma_start(out=memk_f[:, :], in_=memk_d)
            kmem = consts.tile([128, MEM], BF16)
            nc.vector.tensor_copy(kmem[:, :], memk_f[:, :])
            memv_f = io.tile([MEM, 2, DHEAD], F32)
            nc.sync.dma_start(out=memv_f[:, :, :], in_=memv_d)
            nc.vector.tensor_copy(vT[NJT][0][0:MEM, 64:128], memv_f[:, 0, :])
            nc.vector.tensor_copy(vT[NJT][1][0:MEM, 64:128], memv_f[:, 1, :])

            # out-projection lhsT tiles: [hid 128-tile, 64 out channels]
            wout_f = io.tile([128, 4, DHEAD], F32, tag="wout_f")
            for t4 in range(4):
                nc.sync.dma_start(out=wout_f[:, t4, :], in_=wout_d[t4, :, :])
            wout = consts.tile([128, 4, DHEAD], BF16, tag="wout")
            nc.vector.tensor_copy(wout[:, :, :], wout_f[:, :, :])

            # ------------- per-chunk prep: RMSNorm + q/k/vT ----------------
            qs, ks = [None] * NCH, [None] * NCH

            def prep_chunk(ci):
                c0, cw = CHUNKS[ci]
                xsq0 = pP.tile([128, 512], F32R, tag="xsq0", name=f"xsq0_{ci}")
                xsq1 = pP.tile([128, 512], F32R, tag="xsq1", name=f"xsq1_{ci}")
                nc.vector.tensor_mul(xsq0[:, 0:cw], xs[0][ci][:, :], xs[0][ci][:, :])
                nc.vector.tensor_mul(xsq1[:, 0:cw], xs[1][ci][:, :], xs[1][ci][:, :])
                ssq = ps_a.tile([128, 512], F32, tag="a0", name=f"ssq_{ci}")
                nc.tensor.matmul(ssq[:, 0:cw], ones_r[:, :],
                                 xsq0[:, 0:cw], start=True, stop=False)
                nc.tensor.matmul(ssq[:, 0:cw], ones_r[:, :],
                                 xsq1[:, 0:cw], start=False, stop=True)
                sinv = pP.tile([128, 512], F32, tag="sinv", name=f"sinv_{ci}")
                nc.scalar.activation(sinv[:, 0:cw], ssq[:, 0:cw], SQRT,
                                     scale=1.0 / 256.0)
                nc.vector.reciprocal_approx_fast(sinv[:, 0:cw], sinv[:, 0:cw])
                # xn = x * sinv (RMSNorm applied up front; gamma+1 and the
                # q-scale are folded into wqkv host-side)
                xr0 = pP.tile([128, 512], F32R, tag="xn0", name=f"xr0_{ci}")
                xr1 = pP.tile([128, 512], F32R, tag="xn1", name=f"xr1_{ci}")
                nc.vector.tensor_mul(xr0[:, 0:cw], xs[0][ci][:, :], sinv[:, 0:cw])
                nc.vector.tensor_mul(xr1[:, 0:cw], xs[1][ci][:, :], sinv[:, 0:cw])
                xrs = [xr0, xr1]

                qc = big.tile([128, cw], BF16, tag=f"q{ci}", name=f"q_{ci}")
                kc = big.tile([128, cw], BF16, tag=f"k{ci}", name=f"k_{ci}")
                qs[ci], ks[ci] = qc, kc
                for m, dst in ((1, kc), (0, qc)):
                    qp = ps_a.tile([128, 512], F32, tag="a0", name=f"qk_{ci}_{m}")
                    for kt in range(2):
                        nc.tensor.matmul(
                            qp[:, 0:cw],
                            wq[:, kt, m * 128:(m + 1) * 128],
                            xrs[kt][:, 0:cw],
                            start=(kt == 0), stop=(kt == 1),
                        )
                    nc.vector.tensor_copy(dst[:, :], qp[:, 0:cw])
                # v^T directly: out[pos, 2*dhead] = xn_tile^T @ w_v
                for jl in range(cw // 128):
                    jt = c0 // 128 + jl
                    vp = ps_a.tile([128, 512], F32, tag="a1", name=f"vp_{jt}")
                    for kt in range(2):
                        nc.tensor.matmul(
                            vp[:, 0:128],
                            xrs[kt][:, jl * 128:(jl + 1) * 128],
                            wq[:, kt, 256:384],
                            start=(kt == 0), stop=(kt == 1),
                        )
                    nc.vector.tensor_copy(vT[jt][0][:, 64:128], vp[:, 0:64])
                    nc.vector.tensor_copy(vT[jt][1][:, 64:128], vp[:, 64:128])

            accs_by_ci = [None] * NCH

            def attn_part(ci, jts):
                c0, cw = CHUNKS[ci]
                if accs_by_ci[ci] is None:
                    accA = ps_a.tile([128, 512], F32, tag="a0", name=f"accA_{ci}")
                    accB = ps_a.tile([128, 512], F32, tag="a1", name=f"accB_{ci}")
                    accs_by_ci[ci] = [accA, accB]
                accA, accB = accs_by_ci[ci]
                for jt in jts:
                    s_ps = ps_s.tile([128, 2, 512], F32, tag="s",
                                     name=f"s_{ci}_{jt}")
                    if jt < NJT:
                        km = 128
                        klhs = [ks[jt // 4][64 * h:64 * h + 64,
                                            (jt % 4) * 128:(jt % 4) * 128 + 128]
                                for h in range(2)]
                    else:
                        km = MEM
                        klhs = [kmem[64 * h:64 * h + 64, :] for h in range(2)]
                    for h in range(2):
                        nc.tensor.matmul(
                            s_ps[0:km, h, 0:cw],
                            klhs[h],
                            qs[ci][64 * h:64 * h + 64, :],
                            start=True, stop=True,
                        )
                    P = pP.tile([128, 2, 512], BF16, tag="P", name=f"P_{ci}_{jt}")
                    nc.scalar.activation(P[0:km, :, 0:cw], s_ps[0:km, :, 0:cw], EXP)
                    for h, acc in ((0, accA), (1, accB)):
                        nc.tensor.matmul(
                            acc[:, 0:cw],
                            vT[jt][h][0:km, :],
                            P[0:km, h, 0:cw],
                            start=(jt == 0), stop=(jt == NJT),
                            skip_group_check=True,
                        )

            def finish_chunk(ci):
                c0, cw = CHUNKS[ci]
                accs = accs_by_ci[ci]
                rb = pP.tile([128, 2, 512], F32, tag="rb", name=f"rb_{ci}")
                # normalized attention output: oT2 [rows 64:128, head, q]
                oT2 = pP.tile([128, 2, 512], BF16, tag="oT", name=f"oT_{ci}")
                for h in range(2):
                    nc.vector.reciprocal_approx_fast(
                        rec[0:1, h, 0:cw], accs[h][0:1, 0:cw])
                    nc.gpsimd.partition_broadcast(rb[:, h, 0:cw], rec[0:1, h, 0:cw])
                    nc.vector.tensor_mul(
                        oT2[64:128, h, 0:cw], accs[h][64:128, 0:cw],
                        rb[64:128, h, 0:cw])
                nc.sync.dma_start(out=bis[ci][:, :, :], in_=oT2[64:128, :, 0:cw])
                nc.gpsimd.collective_compute(
                    "AllGather", mybir.AluOpType.bypass,
                    replica_groups=GROUPS,
                    ins=[bis[ci][:, :, :].opt()],
                    outs=[bos[ci][:, :, :].opt()],
                )
                # gather the batch's full hidden state, project locally:
                # rank r's heads land on partitions 0:64 / 64:128 of rhs
                # tile r, matching wout's [r*128 + h*64 + d] hidden order.
                g = gth.tile([128, 4, 512], BF16, tag="g", name=f"g_{ci}")
                for t4 in range(4):
                    for h in range(2):
                        eng = [nc.sync, nc.scalar][(2 * t4 + h) % 2]
                        eng.dma_start(
                            out=g[64 * h:64 * h + 64, t4, 0:cw],
                            in_=bos[ci][64 * t4:64 * t4 + 64, h, :])
                op = ps_a.tile([128, 512], F32, tag="a1", name=f"op_{ci}")
                for t4 in range(4):
                    nc.tensor.matmul(
                        op[0:64, 0:cw],
                        wout[:, t4, :],
                        g[:, t4, 0:cw],
                        start=(t4 == 0), stop=(t4 == 3),
                    )
                osb = pP.tile([64, 512], F32, tag="osb", name=f"osb_{ci}")
                nc.vector.tensor_copy(osb[:, 0:cw], op[0:64, 0:cw])
                nc.sync.dma_start(out=out_d[:, c0:c0 + cw], in_=osb[:, 0:cw])

            bis, bos = [], []
            for ci, (c0, cw) in enumerate(CHUNKS):
                bis.append(dram.tile([64, 2, cw], BF16, tag=f"bi{ci}",
                                     name=f"bi_{ci}"))
                bos.append(dram.tile([4 * 64, 2, cw], BF16, tag=f"bo{ci}",
                                     name=f"bo_{ci}"))
            rec = io.tile([1, 2, 512], F32, tag="rec")

            for ci in range(NCH):
                prep_chunk(ci)
            for ci in range(NCH):
                lo = 0 if ci == 0 else 2
                attn_part(ci, range(lo, NJT + 1))
                if ci + 1 < NCH:
                    attn_part(ci + 1, range(0, 2))
                finish_chunk(ci)
    nc.compile()
    return nc


_NC = None
_last_in_maps = None


def _get_nc():
    global _NC
    if _NC is None:
        _NC = build()
    return _NC


def make_in_maps(x, gamma, mem_kv, w_qkv, w_out):
    x = np.asarray(x, np.float32)
    gamma = np.asarray(gamma, np.float32).reshape(DIM)
    mem_kv = np.asarray(mem_kv, np.float32)
    w_qkv = np.asarray(w_qkv, np.float32)
    w_out = np.asarray(w_out, np.float32)

    g1 = 1.0 + gamma  # [256]
    scale = DHEAD ** -0.5
    in_maps = []
    for core in range(8):
        b, g = core // 4, core % 4
        hA, hB = 2 * g, 2 * g + 1
        blocks = []
        for t in range(3):  # q, k, v
            for h in (hA, hB):
                wblk = w_qkv[t * HID + h * DHEAD: t * HID + (h + 1) * DHEAD, :]
                if t == 0:
                    wblk = wblk * scale
                blocks.append(wblk.T)  # [256, 64]
        wqkvT = np.concatenate(blocks, axis=1) * g1[:, None]  # [256, 384]
        memk = np.concatenate(
            [mem_kv[0, hA].T, mem_kv[0, hB].T], axis=0)  # [128, 4]
        memv = np.stack([mem_kv[1, hA], mem_kv[1, hB]], axis=1)  # [4, 2, 64]
        # local out-projection: this core's 64 output channels against the
        # full 512-dim hidden state, as 4 lhsT tiles [128 hid, 64 oc]
        woutT = np.ascontiguousarray(
            w_out[64 * g:64 * g + 64, :].T.reshape(4, 128, DHEAD))
        in_maps.append({
            "x": np.ascontiguousarray(x[b].reshape(DIM, N)),
            "wqkv": np.ascontiguousarray(wqkvT),
            "memk": np.ascontiguousarray(memk),
            "memv": np.ascontiguousarray(memv),
            "wout": woutT,
        })
    return in_maps


def kernel(x, gamma, mem_kv, w_qkv, w_out):
    global _last_in_maps
    in_maps = make_in_maps(x, gamma, mem_kv, w_qkv, w_out)
    _last_in_maps = in_maps
    nc = _get_nc()
    res = run_bass_kernel_spmd(nc, in_maps, core_ids=list(range(8)))
    out = np.empty((2, DIM, N), np.float32)
    for core in range(8):
        b, g = core // 4, core % 4
        out[b, 64 * g:64 * g + 64, :] = res.results[core]["out"]
    return out.reshape(2, DIM, 48, 48)


# revision 24
# speedup vs baseline: 1.1061x; 1.1061x over previous
"""Trainium2 Bass kernel for nn_Attention (dense transformer attention block).

Reference computation (shapes fixed):
  x [2, 256, 48, 48] -> RMSNorm over channels -> 1x1 conv to qkv (8 heads, 64 dhead)
  -> prepend 4 learnable mem kv tokens -> softmax attention -> 1x1 conv out [2, 256, 48, 48]

Sharding: 8 cores = 2 batches x 4 head-pairs. Core c handles batch c//4 and
heads (2g, 2g+1), g = c%4. Each core computes its heads' normalized attention
output oT [128 = 2x64 dhead, q]; per query-chunk the 4 cores of a batch
AllGather their oT (bf16) and every core applies the out-projection locally
for its own 64 output channels -> no reduce collective, tiny tail.

Numerics: qkv projection in float32r; attention matmuls bf16 with fp32 psum.
Layout highlights:
  - x, xn in [channel, pos]; RMSNorm scale sinv broadcast to all partitions
    via all-ones-lhsT matmul of x^2; sinv folded into xn before projections.
  - q/k in [dhead(2 heads packed), pos]; sim row-packed (head A on PE rows
    0-63, head B on 64-127, concurrent via row groups), scores S^T [key, q].
  - v^T computed directly by matmul (lhsT = xn pos-tile, rhs = w_v cols),
    no PE transposes.
  - attn@v per head: lhsT [ones | zeros*63 | v] -> denom on psum row 0,
    out^T on rows 64:128 (both heads identical; no partition-crossing ops).
    Normalized outputs go out as oT2 [rows 64:128, head, q]; the AllGather
    payload is [64, 2, q] per rank and the gather-back DMAs place each
    rank's two heads onto partitions 0:64 / 64:128 of the rhs tiles.
"""
import numpy as np

import concourse.mybir as mybir
import concourse.tile as tile
from concourse import bacc
from concourse.bass_utils import run_bass_kernel_spmd

F32 = mybir.dt.float32
F32R = mybir.dt.float32r
BF16 = mybir.dt.bfloat16
EXP = mybir.ActivationFunctionType.Exp
LN = mybir.ActivationFunctionType.Ln
LN16 = 2.772588722239781  # ln(16)

DIM = 256
HEADS = 8
DHEAD = 64
MEM = 4
HID = 512
N = 48 * 48          # 2304 image positions
NJT = N // 128       # 18 image j-tiles; j-tile NJT holds the 4 mem tokens
GROUPS = [[0, 1, 2, 3], [4, 5, 6, 7]]

# i-chunks of the query axis, in processing order (small chunk first for a
# fast pipeline ramp; offsets are arbitrary)
CHUNKS = [(2048, 256), (0, 512), (512, 512), (1024, 512), (1536, 512)]
# image key-tile jt -> (chunk index, tile offset within chunk)
KEYTILE = {}
# j-tile processing order: key tiles of earliest-prepped chunks first, so
# attention can start as soon as chunk 0's prep lands; mem tokens last.
JT_ORDER = []
for _ci, (_c0, _cw) in enumerate(CHUNKS):
    for _jl in range(_cw // 128):
        KEYTILE[_c0 // 128 + _jl] = (_ci, _jl)
        JT_ORDER.append(_c0 // 128 + _jl)
JT_ORDER.append(N // 128)  # mem j-tile


def build():
    nc = bacc.Bacc("TRN2", target_bir_lowering=False, debug=False,
                   enable_asserts=True, num_devices=8)
    x_d = nc.dram_tensor("x", [DIM, N], F32, kind="ExternalInput").ap()
    wqkv_d = nc.dram_tensor("wqkv", [DIM, 384], F32, kind="ExternalInput").ap()
    memk_d = nc.dram_tensor("memk", [128, MEM], F32, kind="ExternalInput").ap()
    memv_d = nc.dram_tensor("memv", [MEM, 2, DHEAD], F32, kind="ExternalInput").ap()
    wout_d = nc.dram_tensor("wout", [4, 128, DHEAD], F32, kind="ExternalInput").ap()
    out_d = nc.dram_tensor("out", [DHEAD, N], F32, kind="ExternalOutput").ap()

    NCH = len(CHUNKS)
    with tile.TileContext(nc) as tc:
        with (
            tc.tile_pool(name="consts", bufs=1) as consts,
            tc.tile_pool(name="big", bufs=1) as big,
            tc.tile_pool(name="io", bufs=2) as io,
            tc.tile_pool(name="pP", bufs=4) as pP,
            tc.tile_pool(name="gth", bufs=5) as gth,
            tc.tile_pool(name="ps_s", bufs=2, space="PSUM") as ps_s,
            tc.tile_pool(name="ps_a", bufs=2, space="PSUM") as ps_a,
            tc.tile_pool(name="dram", bufs=1, space="DRAM") as dram,
        ):
            # ---------------- constants ----------------
            ones_f = consts.tile([128, 1], F32)
            nc.vector.memset(ones_f[:, :], 1.0)
            zeros_f = consts.tile([128, 1], F32)
            nc.vector.memset(zeros_f[:, :], 0.0)
            ones_r = consts.tile([128, 128], F32R)
            nc.vector.tensor_copy(ones_r[:, :], ones_f[:, :].to_broadcast((128, 128)))
            ln16c = consts.tile([128, 1], F32)
            nc.vector.memset(ln16c[:, :], LN16)

            # ---------------- warmups ----------------
            # first collective on a NEFF pays ~30-60us firmware cold start;
            # absorb it behind the compute phase with a tiny dummy AllGather
            # (same kind as the real collectives).
            warm_sb = consts.tile([1, 32], F32)
            nc.vector.memset(warm_sb[:, :], 1.0)
            wi = dram.tile([1, 32], F32, tag="wi")
            wo = dram.tile([4, 32], F32, tag="wo")
            nc.sync.dma_start(out=wi[:, :], in_=warm_sb[:, :])
            nc.gpsimd.collective_compute(
                "AllGather", mybir.AluOpType.bypass,
                replica_groups=GROUPS,
                ins=[wi[:, :].opt()],
                outs=[wo[:, :].opt()],
            )
            # touch Ln so the single activation table set (natural_log_exp)
            # loads during the input-DMA head phase, off the critical path.
            warm_act = consts.tile([1, 8], F32)
            nc.scalar.activation(warm_act[:, :], warm_sb[0:1, 0:8], LN)

            # ---------------- vT tile skeletons ----------------
            # per (head, jt): [key, 128]: col 0 = ones, 1:64 zeros,
            # 64:128 = v^T.  jt == NJT holds the 4 mem tokens on rows 0:4.
            vT = [[None, None] for _ in range(NJT + 1)]
            for jt in range(NJT + 1):
                for h in range(2):
                    t = big.tile([128, 128], BF16, tag=f"vT{h}_{jt}")
                    vT[jt][h] = t
                    nc.vector.tensor_copy(
                        t[:, 0:1], ones_f[:, :].to_broadcast((128, 1)))
                    nc.vector.tensor_copy(
                        t[:, 1:64], zeros_f[:, :].to_broadcast((128, 63)))

            # ---------------- load inputs ----------------
            xs = [[None] * NCH, [None] * NCH]
            dma_engines = [nc.sync, nc.scalar, nc.gpsimd]
            for ci, (c0, cw) in enumerate(CHUNKS):
                for kt in range(2):
                    t = big.tile([128, cw], F32, tag=f"x{kt}_{ci}")
                    xs[kt][ci] = t
                    eng = dma_engines[(2 * ci + kt) % 3]
                    eng.dma_start(
                        out=t[:, :], in_=x_d[128 * kt:128 * kt + 128, c0:c0 + cw])

            wq_f = io.tile([128, 2, 384], F32)
            nc.sync.dma_start(out=wq_f[:, 0, :], in_=wqkv_d[0:128, :])
            nc.sync.dma_start(out=wq_f[:, 1, :], in_=wqkv_d[128:256, :])
            wq = consts.tile([128, 2, 384], F32R)
            nc.vector.tensor_copy(wq[:, :, :], wq_f[:, :, :])

            memk_f = io.tile([128, MEM], F32)
            nc.sync.dma_start(out=memk_f[:, :], in_=memk_d)
            kmem = consts.tile([128, MEM], BF16)
            nc.vector.tensor_copy(kmem[:, :], memk_f[:, :])
            memv_f = io.tile([MEM, 2, DHEAD], F32)
            nc.sync.dma_start(out=memv_f[:, :, :], in_=memv_d)
            nc.vector.tensor_copy(vT[NJT][0][0:MEM, 64:128], memv_f[:, 0, :])
            nc.vector.tensor_copy(vT[NJT][1][0:MEM, 64:128], memv_f[:, 1, :])

            # out-projection lhsT tiles: [hid 128-tile, 64 out channels]
            wout_f = io.tile([128, 4, DHEAD], F32, tag="wout_f")
            for t4 in range(4):
                nc.sync.dma_start(out=wout_f[:, t4, :], in_=wout_d[t4, :, :])
            wout = consts.tile([128, 4, DHEAD], BF16, tag="wout")
            nc.vector.tensor_copy(wout[:, :, :], wout_f[:, :, :])

            # ------------- per-chunk prep: RMSNorm + q/k/vT ----------------
            qs, ks = [None] * NCH, [None] * NCH

            def prep_chunk(ci):
                c0, cw = CHUNKS[ci]
                xsq0 = pP.tile([128, 512], F32R, tag="xsq0", name=f"xsq0_{ci}")
                xsq1 = pP.tile([128, 512], F32R, tag="xsq1", name=f"xsq1_{ci}")
                nc.vector.tensor_mul(xsq0[:, 0:cw], xs[0][ci][:, :], xs[0][ci][:, :])
                nc.vector.tensor_mul(xsq1[:, 0:cw], xs[1][ci][:, :], xs[1][ci][:, :])
                ssq = ps_a.tile([128, 512], F32, tag="a0", name=f"ssq_{ci}")
                nc.tensor.matmul(ssq[:, 0:cw], ones_r[:, :],
                                 xsq0[:, 0:cw], start=True, stop=False)
                nc.tensor.matmul(ssq[:, 0:cw], ones_r[:, :],
                                 xsq1[:, 0:cw], start=False, stop=True)
                # sinv = 16/sqrt(ssq) = exp(-0.5*ln(ssq) + ln 16); Ln and
                # Exp share one activation table set -> no mid-kernel loads.
                lns = pP.tile([128, 512], F32, tag="lns", name=f"lns_{ci}")
                nc.scalar.activation(lns[:, 0:cw], ssq[:, 0:cw], LN)
                sinv = pP.tile([128, 512], F32, tag="sinv", name=f"sinv_{ci}")
                nc.scalar.activation(sinv[:, 0:cw], lns[:, 0:cw], EXP,
                                     scale=-0.5, bias=ln16c[:, :])
                # xn = x * sinv (RMSNorm applied up front; gamma+1 and the
                # q-scale are folded into wqkv host-side)
                xr0 = pP.tile([128, 512], F32R, tag="xn0", name=f"xr0_{ci}")
                xr1 = pP.tile([128, 512], F32R, tag="xn1", name=f"xr1_{ci}")
                nc.vector.tensor_mul(xr0[:, 0:cw], xs[0][ci][:, :], sinv[:, 0:cw])
                nc.vector.tensor_mul(xr1[:, 0:cw], xs[1][ci][:, :], sinv[:, 0:cw])
                xrs = [xr0, xr1]

                qc = big.tile([128, cw], BF16, tag=f"q{ci}", name=f"q_{ci}")
                kc = big.tile([128, cw], BF16, tag=f"k{ci}", name=f"k_{ci}")
                qs[ci], ks[ci] = qc, kc
                for m, dst in ((1, kc), (0, qc)):
                    qp = ps_a.tile([128, 512], F32, tag="a0", name=f"qk_{ci}_{m}")
                    for kt in range(2):
                        nc.tensor.matmul(
                            qp[:, 0:cw],
                            wq[:, kt, m * 128:(m + 1) * 128],
                            xrs[kt][:, 0:cw],
                            start=(kt == 0), stop=(kt == 1),
                        )
                    nc.vector.tensor_copy(dst[:, :], qp[:, 0:cw])
                # v^T directly: out[pos, 2*dhead] = xn_tile^T @ w_v
                for jl in range(cw // 128):
                    jt = c0 // 128 + jl
                    vp = ps_a.tile([128, 512], F32, tag="a1", name=f"vp_{jt}")
                    for kt in range(2):
                        nc.tensor.matmul(
                            vp[:, 0:128],
                            xrs[kt][:, jl * 128:(jl + 1) * 128],
                            wq[:, kt, 256:384],
                            start=(kt == 0), stop=(kt == 1),
                        )
                    nc.vector.tensor_copy(vT[jt][0][:, 64:128], vp[:, 0:64])
                    nc.vector.tensor_copy(vT[jt][1][:, 64:128], vp[:, 64:128])

            accs_by_ci = [None] * NCH

            def attn_part(ci, jts):
                c0, cw = CHUNKS[ci]
                if accs_by_ci[ci] is None:
                    accA = ps_a.tile([128, 512], F32, tag="a0", name=f"accA_{ci}")
                    accB = ps_a.tile([128, 512], F32, tag="a1", name=f"accB_{ci}")
                    accs_by_ci[ci] = [accA, accB]
                accA, accB = accs_by_ci[ci]
                for jt in (JT_ORDER[j] for j in jts):
                    s_ps = ps_s.tile([128, 2, 512], F32, tag="s",
                                     name=f"s_{ci}_{jt}")
                    if jt < NJT:
                        km = 128
                        kci, kjl = KEYTILE[jt]
                        klhs = [ks[kci][64 * h:64 * h + 64,
                                        kjl * 128:kjl * 128 + 128]
                                for h in range(2)]
                    else:
                        km = MEM
                        klhs = [kmem[64 * h:64 * h + 64, :] for h in range(2)]
                    for h in range(2):
                        nc.tensor.matmul(
                            s_ps[0:km, h, 0:cw],
                            klhs[h],
                            qs[ci][64 * h:64 * h + 64, :],
                            start=True, stop=True,
                        )
                    P = pP.tile([128, 2, 512], BF16, tag="P", name=f"P_{ci}_{jt}")
                    nc.scalar.activation(P[0:km, :, 0:cw], s_ps[0:km, :, 0:cw], EXP)
                    for h, acc in ((0, accA), (1, accB)):
                        nc.tensor.matmul(
                            acc[:, 0:cw],
                            vT[jt][h][0:km, :],
                            P[0:km, h, 0:cw],
                            start=(jt == JT_ORDER[0]),
                            stop=(jt == JT_ORDER[-1]),
                            skip_group_check=True,
                        )

            gs = [None] * NCH

            def send_chunk(ci):
                c0, cw = CHUNKS[ci]
                accs = accs_by_ci[ci]
                rb = pP.tile([128, 2, 512], F32, tag="rb", name=f"rb_{ci}")
                # normalized attention output: oT2 [rows 64:128, head, q]
                oT2 = pP.tile([128, 2, 512], BF16, tag="oT", name=f"oT_{ci}")
                for h in range(2):
                    nc.vector.reciprocal_approx_fast(
                        rec[0:1, h, 0:cw], accs[h][0:1, 0:cw])
                    nc.gpsimd.partition_broadcast(rb[:, h, 0:cw], rec[0:1, h, 0:cw])
                    nc.vector.tensor_mul(
                        oT2[64:128, h, 0:cw], accs[h][64:128, 0:cw],
                        rb[64:128, h, 0:cw])
                nc.sync.dma_start(out=bis[ci][:, :, :], in_=oT2[64:128, :, 0:cw])
                nc.gpsimd.collective_compute(
                    "AllGather", mybir.AluOpType.bypass,
                    replica_groups=GROUPS,
                    ins=[bis[ci][:, :, :].opt()],
                    outs=[bos[ci][:, :, :].opt()],
                )
                # gather the batch's full hidden state: rank r's heads land
                # on partitions 0:64 / 64:128 of rhs tile r, matching wout's
                # [r*128 + h*64 + d] hidden order.
                g = gth.tile([128, 4, 512], BF16, tag="g", name=f"g_{ci}")
                gs[ci] = g
                for t4 in range(4):
                    for h in range(2):
                        eng = [nc.sync, nc.gpsimd][(2 * t4 + h) % 2]
                        eng.dma_start(
                            out=g[64 * h:64 * h + 64, t4, 0:cw],
                            in_=bos[ci][64 * t4:64 * t4 + 64, h, :])

            def proj_chunk(ci):
                # deferred so the PE queue never stalls on a collective
                # mid-kernel; only the last chunk's AllGather is exposed.
                c0, cw = CHUNKS[ci]
                g = gs[ci]
                op = ps_a.tile([128, 512], F32, tag="a1", name=f"op_{ci}")
                for t4 in range(4):
                    nc.tensor.matmul(
                        op[0:64, 0:cw],
                        wout[:, t4, :],
                        g[:, t4, 0:cw],
                        start=(t4 == 0), stop=(t4 == 3),
                    )
                osb = pP.tile([64, 512], F32, tag="osb", name=f"osb_{ci}")
                nc.vector.tensor_copy(osb[:, 0:cw], op[0:64, 0:cw])
                nc.sync.dma_start(out=out_d[:, c0:c0 + cw], in_=osb[:, 0:cw])

            bis, bos = [], []
            for ci, (c0, cw) in enumerate(CHUNKS):
                bis.append(dram.tile([64, 2, cw], BF16, tag=f"bi{ci}",
                                     name=f"bi_{ci}"))
                bos.append(dram.tile([4 * 64, 2, cw], BF16, tag=f"bo{ci}",
                                     name=f"bo_{ci}"))
            rec = io.tile([1, 2, 512], F32, tag="rec")

            for ci in range(NCH):
                prep_chunk(ci)
            for ci in range(NCH):
                lo = 0 if ci == 0 else 2
                attn_part(ci, range(lo, NJT + 1))
                if ci + 1 < NCH:
                    attn_part(ci + 1, range(0, 2))
                send_chunk(ci)
            for ci in range(NCH):
                proj_chunk(ci)
    nc.compile()
    return nc


_NC = None
_last_in_maps = None


def _get_nc():
    global _NC
    if _NC is None:
        _NC = build()
    return _NC


def make_in_maps(x, gamma, mem_kv, w_qkv, w_out):
    x = np.asarray(x, np.float32)
    gamma = np.asarray(gamma, np.float32).reshape(DIM)
    mem_kv = np.asarray(mem_kv, np.float32)
    w_qkv = np.asarray(w_qkv, np.float32)
    w_out = np.asarray(w_out, np.float32)

    g1 = 1.0 + gamma  # [256]
    scale = DHEAD ** -0.5
    in_maps = []
    for core in range(8):
        b, g = core // 4, core % 4
        hA, hB = 2 * g, 2 * g + 1
        blocks = []
        for t in range(3):  # q, k, v
            for h in (hA, hB):
                wblk = w_qkv[t * HID + h * DHEAD: t * HID + (h + 1) * DHEAD, :]
                if t == 0:
                    wblk = wblk * scale
                blocks.append(wblk.T)  # [256, 64]
        wqkvT = np.concatenate(blocks, axis=1) * g1[:, None]  # [256, 384]
        memk = np.concatenate(
            [mem_kv[0, hA].T, mem_kv[0, hB].T], axis=0)  # [128, 4]
        memv = np.stack([mem_kv[1, hA], mem_kv[1, hB]], axis=1)  # [4, 2, 64]
        # local out-projection: this core's 64 output channels against the
        # full 512-dim hidden state, as 4 lhsT tiles [128 hid, 64 oc]
        woutT = np.ascontiguousarray(
            w_out[64 * g:64 * g + 64, :].T.reshape(4, 128, DHEAD))
        in_maps.append({
            "x": np.ascontiguousarray(x[b].reshape(DIM, N)),
            "wqkv": np.ascontiguousarray(wqkvT),
            "memk": np.ascontiguousarray(memk),
            "memv": np.ascontiguousarray(memv),
            "wout": woutT,
        })
    return in_maps


def kernel(x, gamma, mem_kv, w_qkv, w_out):
    global _last_in_maps
    in_maps = make_in_maps(x, gamma, mem_kv, w_qkv, w_out)
    _last_in_maps = in_maps
    nc = _get_nc()
    res = run_bass_kernel_spmd(nc, in_maps, core_ids=list(range(8)))
    out = np.empty((2, DIM, N), np.float32)
    for core in range(8):
        b, g = core // 4, core % 4
        out[b, 64 * g:64 * g + 64, :] = res.results[core]["out"]
    return out.reshape(2, DIM, 48, 48)


# revision 28
# speedup vs baseline: 1.2040x; 1.0885x over previous
"""Trainium2 Bass kernel for nn_Attention (dense transformer attention block).

Reference computation (shapes fixed):
  x [2, 256, 48, 48] -> RMSNorm over channels -> 1x1 conv to qkv (8 heads, 64 dhead)
  -> prepend 4 learnable mem kv tokens -> softmax attention -> 1x1 conv out [2, 256, 48, 48]

Sharding: 8 cores = 2 batches x 4 head-pairs. Core c handles batch c//4 and
heads (2g, 2g+1), g = c%4. Each core computes its heads' normalized attention
output oT [128 = 2x64 dhead, q]; per query-chunk the 4 cores of a batch
AllGather their oT (bf16) and every core applies the out-projection locally
for its own 64 output channels -> no reduce collective, tiny tail.

Numerics: qkv projection in float32r; attention matmuls bf16 with fp32 psum.
Layout highlights:
  - x, xn in [channel, pos]; RMSNorm scale sinv broadcast to all partitions
    via all-ones-lhsT matmul of x^2; sinv folded into xn before projections.
  - q/k in [dhead(2 heads packed), pos]; sim row-packed (head A on PE rows
    0-63, head B on 64-127, concurrent via row groups), scores S^T [key, q].
  - v^T computed directly by matmul (lhsT = xn pos-tile, rhs = w_v cols),
    no PE transposes.
  - attn@v per head: lhsT [ones | zeros*63 | v] -> denom on psum row 0,
    out^T on rows 64:128 (both heads identical; no partition-crossing ops).
    Normalized outputs go out as oT2 [rows 64:128, head, q]; the AllGather
    payload is [64, 2, q] per rank and the gather-back DMAs place each
    rank's two heads onto partitions 0:64 / 64:128 of the rhs tiles.
"""
import numpy as np

import concourse.mybir as mybir
import concourse.tile as tile
from concourse import bacc
from concourse.bass_utils import run_bass_kernel_spmd

F32 = mybir.dt.float32
F32R = mybir.dt.float32r
BF16 = mybir.dt.bfloat16
EXP = mybir.ActivationFunctionType.Exp
LN = mybir.ActivationFunctionType.Ln
LN16 = 2.772588722239781  # ln(16)

DIM = 256
HEADS = 8
DHEAD = 64
MEM = 4
HID = 512
N = 48 * 48          # 2304 image positions
NJT = N // 128       # 18 image j-tiles; j-tile NJT holds the 4 mem tokens
GROUPS = [[0, 1, 2, 3], [4, 5, 6, 7]]

# i-chunks of the query axis, in processing order (small chunk first for a
# fast pipeline ramp; offsets are arbitrary)
CHUNKS = [(2048, 256), (0, 512), (512, 512), (1024, 512), (1536, 512)]
# image key-tile jt -> (chunk index, tile offset within chunk)
KEYTILE = {}
# j-tile processing order: key tiles of earliest-prepped chunks first, so
# attention can start as soon as chunk 0's prep lands; mem tokens last.
JT_ORDER = []
for _ci, (_c0, _cw) in enumerate(CHUNKS):
    for _jl in range(_cw // 128):
        KEYTILE[_c0 // 128 + _jl] = (_ci, _jl)
        JT_ORDER.append(_c0 // 128 + _jl)
JT_ORDER.append(N // 128)  # mem j-tile


def _patch_act_tables():
    """Make the table-set picker resolve both Ln and Exp to the combined
    natural_log_exp_and_others set (one load for the whole kernel). Only the
    metadata the picker sees is filtered; set ids and the runtime tables are
    unchanged, so every emitted load still refers to a real superset."""
    from concourse.hw_specs import get_activation_tables as real_gat

    def gat(arch):
        strip = {mybir.ActivationFunctionType.Exp, mybir.ActivationFunctionType.Ln}
        return {
            name: (funcs if name == "natural_log_exp_and_others"
                   else funcs - strip)
            for name, funcs in real_gat(arch).items()
        }

    bacc.get_activation_tables = gat


def build():
    _patch_act_tables()
    nc = bacc.Bacc("TRN2", target_bir_lowering=False, debug=False,
                   enable_asserts=True, num_devices=8)
    x_d = nc.dram_tensor("x", [DIM, N], F32, kind="ExternalInput").ap()
    wqkv_d = nc.dram_tensor("wqkv", [DIM, 384], F32, kind="ExternalInput").ap()
    memk_d = nc.dram_tensor("memk", [128, MEM], F32, kind="ExternalInput").ap()
    memv_d = nc.dram_tensor("memv", [MEM, 2, DHEAD], F32, kind="ExternalInput").ap()
    wout_d = nc.dram_tensor("wout", [4, 128, DHEAD], F32, kind="ExternalInput").ap()
    out_d = nc.dram_tensor("out", [DHEAD, N], F32, kind="ExternalOutput").ap()

    NCH = len(CHUNKS)
    with tile.TileContext(nc) as tc:
        with (
            tc.tile_pool(name="consts", bufs=1) as consts,
            tc.tile_pool(name="big", bufs=1) as big,
            tc.tile_pool(name="io", bufs=2) as io,
            tc.tile_pool(name="pP", bufs=4) as pP,
            tc.tile_pool(name="gth", bufs=5) as gth,
            tc.tile_pool(name="ps_s", bufs=2, space="PSUM") as ps_s,
            tc.tile_pool(name="ps_a", bufs=2, space="PSUM") as ps_a,
            tc.tile_pool(name="dram", bufs=1, space="DRAM") as dram,
        ):
            # ---------------- constants ----------------
            ones_f = consts.tile([128, 1], F32)
            nc.vector.memset(ones_f[:, :], 1.0)
            zeros_f = consts.tile([128, 1], F32)
            nc.vector.memset(zeros_f[:, :], 0.0)
            ones_r = consts.tile([128, 128], F32R)
            nc.vector.tensor_copy(ones_r[:, :], ones_f[:, :].to_broadcast((128, 128)))
            ln16c = consts.tile([128, 1], F32)
            nc.vector.memset(ln16c[:, :], LN16)

            # ---------------- warmups ----------------
            # first collective on a NEFF pays ~30-60us firmware cold start;
            # absorb it behind the compute phase with a tiny dummy AllGather
            # (same kind as the real collectives).
            warm_sb = consts.tile([1, 32], F32)
            nc.vector.memset(warm_sb[:, :], 1.0)
            wi = dram.tile([1, 32], F32, tag="wi")
            wo = dram.tile([4, 32], F32, tag="wo")
            nc.sync.dma_start(out=wi[:, :], in_=warm_sb[:, :])
            nc.gpsimd.collective_compute(
                "AllGather", mybir.AluOpType.bypass,
                replica_groups=GROUPS,
                ins=[wi[:, :].opt()],
                outs=[wo[:, :].opt()],
            )
            # touch Ln so the single activation table set (natural_log_exp)
            # loads during the input-DMA head phase, off the critical path.
            warm_act = consts.tile([1, 8], F32)
            nc.scalar.activation(warm_act[:, :], warm_sb[0:1, 0:8], LN)

            # ---------------- vT tile skeletons ----------------
            # per (head, jt): [key, 128]: col 0 = ones, 1:64 zeros,
            # 64:128 = v^T.  jt == NJT holds the 4 mem tokens on rows 0:4.
            vT = [[None, None] for _ in range(NJT + 1)]
            for jt in range(NJT + 1):
                for h in range(2):
                    t = big.tile([128, 128], BF16, tag=f"vT{h}_{jt}")
                    vT[jt][h] = t
                    nc.vector.tensor_copy(
                        t[:, 0:1], ones_f[:, :].to_broadcast((128, 1)))
                    nc.vector.tensor_copy(
                        t[:, 1:64], zeros_f[:, :].to_broadcast((128, 63)))

            # ---------------- load inputs ----------------
            xs = [[None] * NCH, [None] * NCH]
            dma_engines = [nc.sync, nc.scalar, nc.gpsimd]
            for ci, (c0, cw) in enumerate(CHUNKS):
                for kt in range(2):
                    t = big.tile([128, cw], F32, tag=f"x{kt}_{ci}")
                    xs[kt][ci] = t
                    eng = dma_engines[(2 * ci + kt) % 3]
                    eng.dma_start(
                        out=t[:, :], in_=x_d[128 * kt:128 * kt + 128, c0:c0 + cw])

            wq_f = io.tile([128, 2, 384], F32)
            nc.sync.dma_start(out=wq_f[:, 0, :], in_=wqkv_d[0:128, :])
            nc.sync.dma_start(out=wq_f[:, 1, :], in_=wqkv_d[128:256, :])
            wq = consts.tile([128, 2, 384], F32R)
            nc.vector.tensor_copy(wq[:, :, :], wq_f[:, :, :])

            memk_f = io.tile([128, MEM], F32)
            nc.sync.dma_start(out=memk_f[:, :], in_=memk_d)
            kmem = consts.tile([128, MEM], BF16)
            nc.vector.tensor_copy(kmem[:, :], memk_f[:, :])
            memv_f = io.tile([MEM, 2, DHEAD], F32)
            nc.sync.dma_start(out=memv_f[:, :, :], in_=memv_d)
            nc.vector.tensor_copy(vT[NJT][0][0:MEM, 64:128], memv_f[:, 0, :])
            nc.vector.tensor_copy(vT[NJT][1][0:MEM, 64:128], memv_f[:, 1, :])

            # out-projection lhsT tiles: [hid 128-tile, 64 out channels]
            wout_f = io.tile([128, 4, DHEAD], F32, tag="wout_f")
            for t4 in range(4):
                nc.sync.dma_start(out=wout_f[:, t4, :], in_=wout_d[t4, :, :])
            wout = consts.tile([128, 4, DHEAD], BF16, tag="wout")
            nc.vector.tensor_copy(wout[:, :, :], wout_f[:, :, :])

            # ------------- per-chunk prep: RMSNorm + q/k/vT ----------------
            qs, ks = [None] * NCH, [None] * NCH

            def prep_chunk(ci):
                c0, cw = CHUNKS[ci]
                xsq0 = pP.tile([128, 512], F32R, tag="xsq0", name=f"xsq0_{ci}")
                xsq1 = pP.tile([128, 512], F32R, tag="xsq1", name=f"xsq1_{ci}")
                nc.vector.tensor_mul(xsq0[:, 0:cw], xs[0][ci][:, :], xs[0][ci][:, :])
                nc.vector.tensor_mul(xsq1[:, 0:cw], xs[1][ci][:, :], xs[1][ci][:, :])
                ssq = ps_a.tile([128, 512], F32, tag="a0", name=f"ssq_{ci}")
                nc.tensor.matmul(ssq[:, 0:cw], ones_r[:, :],
                                 xsq0[:, 0:cw], start=True, stop=False)
                nc.tensor.matmul(ssq[:, 0:cw], ones_r[:, :],
                                 xsq1[:, 0:cw], start=False, stop=True)
                # sinv = 16/sqrt(ssq) = exp(-0.5*ln(ssq) + ln 16); Ln and
                # Exp share one activation table set -> no mid-kernel loads.
                lns = pP.tile([128, 512], F32, tag="lns", name=f"lns_{ci}")
                nc.scalar.activation(lns[:, 0:cw], ssq[:, 0:cw], LN)
                sinv = pP.tile([128, 512], F32, tag="sinv", name=f"sinv_{ci}")
                nc.scalar.activation(sinv[:, 0:cw], lns[:, 0:cw], EXP,
                                     scale=-0.5, bias=ln16c[:, :])
                # xn = x * sinv (RMSNorm applied up front; gamma+1 and the
                # q-scale are folded into wqkv host-side)
                xr0 = pP.tile([128, 512], F32R, tag="xn0", name=f"xr0_{ci}")
                xr1 = pP.tile([128, 512], F32R, tag="xn1", name=f"xr1_{ci}")
                nc.vector.tensor_mul(xr0[:, 0:cw], xs[0][ci][:, :], sinv[:, 0:cw])
                nc.vector.tensor_mul(xr1[:, 0:cw], xs[1][ci][:, :], sinv[:, 0:cw])
                xrs = [xr0, xr1]

                qc = big.tile([128, cw], BF16, tag=f"q{ci}", name=f"q_{ci}")
                kc = big.tile([128, cw], BF16, tag=f"k{ci}", name=f"k_{ci}")
                qs[ci], ks[ci] = qc, kc
                for m, dst in ((1, kc), (0, qc)):
                    qp = ps_a.tile([128, 512], F32, tag="a0", name=f"qk_{ci}_{m}")
                    for kt in range(2):
                        nc.tensor.matmul(
                            qp[:, 0:cw],
                            wq[:, kt, m * 128:(m + 1) * 128],
                            xrs[kt][:, 0:cw],
                            start=(kt == 0), stop=(kt == 1),
                        )
                    nc.vector.tensor_copy(dst[:, :], qp[:, 0:cw])
                # v^T directly: out[pos, 2*dhead] = xn_tile^T @ w_v
                for jl in range(cw // 128):
                    jt = c0 // 128 + jl
                    vp = ps_a.tile([128, 512], F32, tag="a1", name=f"vp_{jt}")
                    for kt in range(2):
                        nc.tensor.matmul(
                            vp[:, 0:128],
                            xrs[kt][:, jl * 128:(jl + 1) * 128],
                            wq[:, kt, 256:384],
                            start=(kt == 0), stop=(kt == 1),
                        )
                    nc.vector.tensor_copy(vT[jt][0][:, 64:128], vp[:, 0:64])
                    nc.vector.tensor_copy(vT[jt][1][:, 64:128], vp[:, 64:128])

            accs_by_ci = [None] * NCH

            def attn_part(ci, jts):
                c0, cw = CHUNKS[ci]
                if accs_by_ci[ci] is None:
                    accA = ps_a.tile([128, 512], F32, tag="a0", name=f"accA_{ci}")
                    accB = ps_a.tile([128, 512], F32, tag="a1", name=f"accB_{ci}")
                    accs_by_ci[ci] = [accA, accB]
                accA, accB = accs_by_ci[ci]
                for jt in (JT_ORDER[j] for j in jts):
                    s_ps = ps_s.tile([128, 2, 512], F32, tag="s",
                                     name=f"s_{ci}_{jt}")
                    if jt < NJT:
                        km = 128
                        kci, kjl = KEYTILE[jt]
                        klhs = [ks[kci][64 * h:64 * h + 64,
                                        kjl * 128:kjl * 128 + 128]
                                for h in range(2)]
                    else:
                        km = MEM
                        klhs = [kmem[64 * h:64 * h + 64, :] for h in range(2)]
                    for h in range(2):
                        nc.tensor.matmul(
                            s_ps[0:km, h, 0:cw],
                            klhs[h],
                            qs[ci][64 * h:64 * h + 64, :],
                            start=True, stop=True,
                        )
                    P = pP.tile([128, 2, 512], BF16, tag="P", name=f"P_{ci}_{jt}")
                    nc.scalar.activation(P[0:km, :, 0:cw], s_ps[0:km, :, 0:cw], EXP)
                    for h, acc in ((0, accA), (1, accB)):
                        nc.tensor.matmul(
                            acc[:, 0:cw],
                            vT[jt][h][0:km, :],
                            P[0:km, h, 0:cw],
                            start=(jt == JT_ORDER[0]),
                            stop=(jt == JT_ORDER[-1]),
                            skip_group_check=True,
                        )

            def send_chunk(ci):
                c0, cw = CHUNKS[ci]
                accs = accs_by_ci[ci]
                rb = pP.tile([128, 2, 512], F32, tag="rb", name=f"rb_{ci}")
                # normalized attention output: oT2 [rows 64:128, head, q]
                oT2 = pP.tile([128, 2, 512], BF16, tag="oT", name=f"oT_{ci}")
                for h in range(2):
                    nc.vector.reciprocal_approx_fast(
                        rec[0:1, h, 0:cw], accs[h][0:1, 0:cw])
                    nc.gpsimd.partition_broadcast(rb[:, h, 0:cw], rec[0:1, h, 0:cw])
                    nc.vector.tensor_mul(
                        oT2[64:128, h, 0:cw], accs[h][64:128, 0:cw],
                        rb[64:128, h, 0:cw])
                nc.sync.dma_start(out=bis[ci][:, :, :], in_=oT2[64:128, :, 0:cw])
                nc.gpsimd.collective_compute(
                    "AllGather", mybir.AluOpType.bypass,
                    replica_groups=GROUPS,
                    ins=[bis[ci][:, :, :].opt()],
                    outs=[bos[ci][:, :, :].opt()],
                )
            def proj_chunk(ci):
                # deferred (incl. the gather-back DMAs, which block their
                # issue queue while waiting on the collective) so no engine
                # queue ever stalls on a collective mid-kernel; only the
                # last chunk's AllGather is exposed at the tail.
                c0, cw = CHUNKS[ci]
                # rank r's heads land on partitions 0:64 / 64:128 of rhs
                # tile r, matching wout's [r*128 + h*64 + d] hidden order.
                g = gth.tile([128, 4, 512], BF16, tag="g", name=f"g_{ci}")
                for t4 in range(4):
                    for h in range(2):
                        eng = [nc.sync, nc.scalar][(2 * t4 + h) % 2]
                        eng.dma_start(
                            out=g[64 * h:64 * h + 64, t4, 0:cw],
                            in_=bos[ci][64 * t4:64 * t4 + 64, h, :])
                op = ps_a.tile([128, 512], F32, tag="a1", name=f"op_{ci}")
                for t4 in range(4):
                    nc.tensor.matmul(
                        op[0:64, 0:cw],
                        wout[:, t4, :],
                        g[:, t4, 0:cw],
                        start=(t4 == 0), stop=(t4 == 3),
                    )
                osb = pP.tile([64, 512], F32, tag="osb", name=f"osb_{ci}")
                nc.vector.tensor_copy(osb[:, 0:cw], op[0:64, 0:cw])
                nc.sync.dma_start(out=out_d[:, c0:c0 + cw], in_=osb[:, 0:cw])

            bis, bos = [], []
            for ci, (c0, cw) in enumerate(CHUNKS):
                bis.append(dram.tile([64, 2, cw], BF16, tag=f"bi{ci}",
                                     name=f"bi_{ci}"))
                bos.append(dram.tile([4 * 64, 2, cw], BF16, tag=f"bo{ci}",
                                     name=f"bo_{ci}"))
            rec = io.tile([1, 2, 512], F32, tag="rec")

            for ci in range(NCH):
                prep_chunk(ci)
            for ci in range(NCH):
                lo = 0 if ci == 0 else 2
                attn_part(ci, range(lo, NJT + 1))
                if ci + 1 < NCH:
                    attn_part(ci + 1, range(0, 2))
                send_chunk(ci)
            for ci in range(NCH):
                proj_chunk(ci)
    nc.compile()
    return nc


_NC = None
_last_in_maps = None


def _get_nc():
    global _NC
    if _NC is None:
        _NC = build()
    return _NC


def make_in_maps(x, gamma, mem_kv, w_qkv, w_out):
    x = np.asarray(x, np.float32)
    gamma = np.asarray(gamma, np.float32).reshape(DIM)
    mem_kv = np.asarray(mem_kv, np.float32)
    w_qkv = np.asarray(w_qkv, np.float32)
    w_out = np.asarray(w_out, np.float32)

    g1 = 1.0 + gamma  # [256]
    scale = DHEAD ** -0.5
    in_maps = []
    for core in range(8):
        b, g = core // 4, core % 4
        hA, hB = 2 * g, 2 * g + 1
        blocks = []
        for t in range(3):  # q, k, v
            for h in (hA, hB):
                wblk = w_qkv[t * HID + h * DHEAD: t * HID + (h + 1) * DHEAD, :]
                if t == 0:
                    wblk = wblk * scale
                blocks.append(wblk.T)  # [256, 64]
        wqkvT = np.concatenate(blocks, axis=1) * g1[:, None]  # [256, 384]
        memk = np.concatenate(
            [mem_kv[0, hA].T, mem_kv[0, hB].T], axis=0)  # [128, 4]
        memv = np.stack([mem_kv[1, hA], mem_kv[1, hB]], axis=1)  # [4, 2, 64]
        # local out-projection: this core's 64 output channels against the
        # full 512-dim hidden state, as 4 lhsT tiles [128 hid, 64 oc]
        woutT = np.ascontiguousarray(
            w_out[64 * g:64 * g + 64, :].T.reshape(4, 128, DHEAD))
        in_maps.append({
            "x": np.ascontiguousarray(x[b].reshape(DIM, N)),
            "wqkv": np.ascontiguousarray(wqkvT),
            "memk": np.ascontiguousarray(memk),
            "memv": np.ascontiguousarray(memv),
            "wout": woutT,
        })
    return in_maps


def kernel(x, gamma, mem_kv, w_qkv, w_out):
    global _last_in_maps
    in_maps = make_in_maps(x, gamma, mem_kv, w_qkv, w_out)
    _last_in_maps = in_maps
    nc = _get_nc()
    res = run_bass_kernel_spmd(nc, in_maps, core_ids=list(range(8)))
    out = np.empty((2, DIM, N), np.float32)
    for core in range(8):
        b, g = core // 4, core % 4
        out[b, 64 * g:64 * g + 64, :] = res.results[core]["out"]
    return out.reshape(2, DIM, 48, 48)
